# revision 33
# baseline (speedup 1.0000x reference)
"""Trainium2 Bass kernel for BodyConvClothGraphConvolution.

Reference computation (R = C = 8192, D = H = 256):
    X0  = notes @ w                     # (R+C, H)
    top = X0[:R] + weight @ X0[R:]      # (R, H)
    out = concat([relu(top + b), relu(b)*ones(C,H), X0[R:]], axis=0)

Sharding (8 cores, zero cross-core communication):
  - weight rows and cloth notes rows are sharded 8-way (1024 rows/core);
    body notes NB and w/b are replicated.
  - Associativity: per core, W_k @ (NB @ w) is computed as (W_k @ NB) @ w.
    The 8192-deep contraction runs against raw NB (same FLOPs), but the
    h-projection then applies to the small (1024 x 256) partial P = W_k@NB
    instead of the (8192 x 256) X0b — this removes phase 2's replicated
    projection almost entirely. Each core also projects only its OWN 1024
    body rows for the X0[R:] output block.

Per-core kernel (all matmuls bf16 inputs, fp32 PSUM accumulation; on HW the
sustained matmul rate is ~(131ns + N/2.4GHz)/instruction, so everything is
structured as few, wide N=512 matmuls):
  warm:  ~9 dummy matmuls warm the PE HAM clock gate while input DMA lands
  own:   X0ownT[h, c_own] = (NB_own @ w).T   (8 MMs, w stationary)
  main:  PT[d, m] = (W_k @ NB).T: weight streams m-major (all 64 c-blocks
         for m-chunk 0, then m-chunk 1) through 4 persistent PSUM banks
         (2 d-tiles x 2 m-chunks); NB tiles are the stationary operand.
  fold:  topT[h, m] = b + (Xc0 @ w).T + (P @ w).T — 16 MMs on the small P,
         pass-0's fold/relu/store overlap pass 1 of the weight stream.
"""

import numpy as np
import ml_dtypes

R, C, D, H = 8192, 8192, 256, 256
NCORES = 8
MSHARD = R // NCORES          # 1024 cloth rows / weight rows per core
NCT = C // 128                # 64 body-vertex 128-blocks
NDT = D // 128                # 2 contraction tiles over d
NHT = H // 128                # 2 h-tiles
NSLAB = 8                     # weight DMA slabs per m-chunk pass (1MB each)
N_WARM = 5                    # HAM warmup dummy matmuls (the own-block
                              # projection continues the warm-up after them)

BF16 = ml_dtypes.bfloat16

_CACHE = {}


def _build_nc(reps=1, loop_iters=1):
    """Build + compile the SPMD Bass program (same program for all cores).

    reps > 1 statically repeats the whole body; loop_iters > 1 wraps the body
    in a hardware For_i loop. Both are used only by the timing harness to
    isolate per-execution device time by wall-clock slope.
    """
    import concourse.bass as bass
    import concourse.bacc as bacc
    import concourse.tile as tile
    from concourse import mybir

    fp32 = mybir.dt.float32
    bf16 = mybir.dt.bfloat16

    nc = bacc.Bacc("TRN2", target_bir_lowering=False, debug=False,
                   num_devices=NCORES)

    # DRAM I/O (per-core shapes)
    nb_d = nc.dram_tensor("nb", [128, NCT * D], bf16,
                          kind="ExternalInput").ap()
    nbo_d = nc.dram_tensor("nbo", [128, NDT * MSHARD], bf16,
                           kind="ExternalInput").ap()
    nct_d = nc.dram_tensor("nct", [128, NDT * MSHARD], bf16,
                           kind="ExternalInput").ap()
    wt_d = nc.dram_tensor("wt", [128, NDT * H], bf16,
                          kind="ExternalInput").ap()
    b2_d = nc.dram_tensor("b2", [128, NHT], fp32, kind="ExternalInput").ap()
    wpe_d = nc.dram_tensor("wpe", [2 * NSLAB, 128, 8 * 512], bf16,
                           kind="ExternalInput").ap()
    top_d = nc.dram_tensor("topt_out", [NHT, 128, MSHARD], fp32,
                           kind="ExternalOutput").ap()
    x0b_d = nc.dram_tensor("x0b_out", [128, NHT * MSHARD], bf16,
                           kind="ExternalOutput").ap()

    def body(tc, const_pool, wpe_pool, ps2_pool, ps3_pool, out_pool):
        # ---- HAM warmup (PE busy from ~0.1us) ----
        # The matmuls read columns the memset never touches (subtile dep
        # tracking -> no cross-engine wait); garbage inputs are fine, the
        # products are never consumed. ~3.4us of PE activity opens the HAM
        # clock gate (1.2 -> 2.4 GHz) before the first real matmul.
        warm_sb = const_pool.tile([128, 644], bf16)
        nc.vector.memset(warm_sb[:, 0:4], 0.0)
        warm_ps = ps2_pool.tile([128, 512], fp32, name="warm", tag="ps")
        for i in range(N_WARM):
            nc.tensor.matmul(warm_ps[:, :], lhsT=warm_sb[:, 4:132],
                             rhs=warm_sb[:, 132:644], start=True, stop=True)

        wt_sb = const_pool.tile([128, NDT * H], bf16)
        nbo_sb = const_pool.tile([128, NDT * MSHARD], bf16)
        nb_sb = const_pool.tile([128, NCT * D], bf16)
        nct_sb = const_pool.tile([128, NDT * MSHARD], bf16)
        b2_sb = const_pool.tile([128, NHT], fp32)
        pbf_sb = const_pool.tile([128, NDT * MSHARD], bf16)
        x0o_sb = const_pool.tile([128, NHT * MSHARD], bf16)

        nc.sync.dma_start(out=wt_sb[:, :], in_=wt_d[:, :])
        nc.sync.dma_start(out=nbo_sb[:, :], in_=nbo_d[:, :])
        # b2 off the critical preamble, on the scalar HWDGE queue
        nc.scalar.dma_start(out=b2_sb[:, :], in_=b2_d[:, :])
        # NB streams in 8 chunks of 8 c-blocks (0.5MB each) INTERLEAVED with
        # the pass-0 weight slabs: slab s consumes exactly NB chunk s, so
        # this order lets the main stream start ~11us earlier than
        # NB-then-slabs would
        slabs0 = []
        for cc in range(NSLAB):
            nc.sync.dma_start(
                out=nb_sb[:, cc * 8 * D:(cc + 1) * 8 * D],
                in_=nb_d[:, cc * 8 * D:(cc + 1) * 8 * D])
            wslab = wpe_pool.tile([128, 8 * 512], bf16)
            if cc == 0:
                # split the first fetch so the stream unblocks on the
                # first half-slab earlier (subtile deps)
                nc.sync.dma_start(out=wslab[:, 0:4 * 512],
                                  in_=wpe_d[0, :, 0:4 * 512])
                nc.sync.dma_start(out=wslab[:, 4 * 512:8 * 512],
                                  in_=wpe_d[0, :, 4 * 512:8 * 512])
            else:
                nc.sync.dma_start(out=wslab[:, :], in_=wpe_d[cc])
            slabs0.append(wslab)

        # ---- own-block projection: X0ownT[h, c_own] = (NB_own @ w).T ----
        for ht in range(NHT):
            for oc in range(2):
                ps = ps2_pool.tile([128, 512], fp32, name="own", tag="ps")
                for dt in range(NDT):
                    nc.tensor.matmul(
                        ps[:, :],
                        lhsT=wt_sb[:, dt * H + ht * 128:
                                   dt * H + (ht + 1) * 128],
                        rhs=nbo_sb[:, dt * MSHARD + oc * 512:
                                   dt * MSHARD + (oc + 1) * 512],
                        start=(dt == 0), stop=(dt == NDT - 1),
                    )
                dst = x0o_sb[:, ht * MSHARD + oc * 512:
                             ht * MSHARD + (oc + 1) * 512]
                if (ht * 2 + oc) % 2 == 0:
                    nc.vector.tensor_copy(out=dst, in_=ps[:, :])
                else:
                    nc.scalar.copy(out=dst, in_=ps[:, :])

        # X0 own-block store: one contiguous bf16 burst; host upcasts +
        # reshapes. Issued AFTER the projection writes x0o_sb (program order
        # is dep order for Tile) but queued on the sync FIFO behind the
        # pass-0 slab fetches, so it executes past the head DMA crunch.
        nc.sync.dma_start(out=x0b_d[:, :], in_=x0o_sb[:, :])
        # nct feeds the fold stage (~mid-stream); stream it after NB
        nc.sync.dma_start(out=nct_sb[:, :], in_=nct_d[:, :])

        # ---- main: PT[d, m] = (W_k @ NB).T, m-major weight stream ----
        # PSUM bank (dt, mc) accumulates PT[dt*128:(dt+1)*128,
        # mc*512:(mc+1)*512] over all 64 c-blocks; NB block tiles are the
        # stationary operand, the weight slab the N=512 moving operand.
        pT = [ps3_pool.tile([128, 512], fp32, name=f"pT{g}", tag=f"pT{g}")
              for g in range(NDT * 2)]

        psg = {}

        def fold_nct(mc):
            # init topT[ht][mc] accumulators with the cloth term (Xc0 @ w).T
            # — independent of P, so pass 1's init can run early
            psg[mc] = [ps2_pool.tile([128, 512], fp32, name=f"psg{mc}{ht}",
                                     tag="ps") for ht in range(NHT)]
            for dt in range(NDT):
                for ht in range(NHT):
                    nc.tensor.matmul(
                        psg[mc][ht][:, :],
                        lhsT=wt_sb[:, dt * H + ht * 128:
                                   dt * H + (ht + 1) * 128],
                        rhs=nct_sb[:, dt * MSHARD + mc * 512:
                                   dt * MSHARD + (mc + 1) * 512],
                        start=(dt == 0), stop=False,
                    )

        def fold_mc(mc):
            # += (P @ w).T, then relu(+bias) and store
            for dt in range(NDT):
                for ht in range(NHT):
                    nc.tensor.matmul(
                        psg[mc][ht][:, :],
                        lhsT=wt_sb[:, dt * H + ht * 128:
                                   dt * H + (ht + 1) * 128],
                        rhs=pbf_sb[:, dt * MSHARD + mc * 512:
                                   dt * MSHARD + (mc + 1) * 512],
                        start=False, stop=(dt == NDT - 1),
                    )
            # relu in 256-wide halves, one batched store per h-tile; the
            # two tail stores use different HWDGE queues
            for ht in range(NHT):
                o = out_pool.tile([128, 512], fp32, tag="topout")
                for hf in range(2):
                    nc.scalar.activation(
                        o[:, hf * 256:(hf + 1) * 256],
                        psg[mc][ht][:, hf * 256:(hf + 1) * 256],
                        mybir.ActivationFunctionType.Relu,
                        bias=b2_sb[:, ht:ht + 1])
                eng = nc.sync if ht == 0 else nc.scalar
                eng.dma_start(
                    out=top_d[ht, :, mc * 512:(mc + 1) * 512],
                    in_=o[:, :])

        for mc in range(2):
            for cbp in range(NSLAB):
                if mc == 0:
                    wslab = slabs0[cbp]
                else:
                    wslab = wpe_pool.tile([128, 8 * 512], bf16)
                    nc.sync.dma_start(out=wslab[:, :],
                                      in_=wpe_d[mc * NSLAB + cbp])
                if mc == 1 and cbp == 2:
                    # pass-0 P is copied out by now: fold + relu + store
                    # m-chunk 0 while pass 1 streams
                    fold_nct(0)
                    fold_mc(0)
                for j in range(8):
                    ct = cbp * 8 + j
                    for dt in range(NDT):
                        nc.tensor.matmul(
                            pT[dt * 2 + mc][:, :],
                            lhsT=nb_sb[:, ct * D + dt * 128:
                                       ct * D + (dt + 1) * 128],
                            rhs=wslab[:, j * 512:(j + 1) * 512],
                            start=(ct == 0), stop=(ct == NCT - 1),
                        )
            # end of pass: copy this pass's PT banks to bf16 for the fold
            for dt in range(NDT):
                dst = pbf_sb[:, dt * MSHARD + mc * 512:
                             dt * MSHARD + (mc + 1) * 512]
                if dt == 0:
                    nc.vector.tensor_copy(out=dst, in_=pT[dt * 2 + mc][:, :])
                else:
                    nc.scalar.copy(out=dst, in_=pT[dt * 2 + mc][:, :])
        fold_nct(1)
        fold_mc(1)

    with tile.TileContext(nc) as tc:
        with (
            tc.tile_pool(name="const", bufs=1) as const_pool,
            tc.tile_pool(name="wpe", bufs=8) as wpe_pool,
            tc.tile_pool(name="ps2", bufs=4, space="PSUM") as ps2_pool,
            tc.tile_pool(name="ps3", bufs=1, space="PSUM") as ps3_pool,
            tc.tile_pool(name="outs", bufs=4) as out_pool,
        ):
            pools = (const_pool, wpe_pool, ps2_pool, ps3_pool, out_pool)
            if loop_iters > 1:
                with tc.For_i(0, loop_iters, 1,
                              hint_engines=(mybir.EngineType.PE,)):
                    body(tc, *pools)
            else:
                for _rep in range(reps):
                    body(tc, *pools)

    nc.compile()
    return nc


def _get_nc(reps=1, loop_iters=1):
    key = ("nc", reps, loop_iters)
    if key not in _CACHE:
        _CACHE[key] = _build_nc(reps, loop_iters)
    return _CACHE[key]


def _pack_inputs(notes, weight, w, b):
    """Host-side shard + layout + bf16 cast into per-core in_maps."""
    nb = np.ascontiguousarray(notes[R:]).astype(BF16)      # (C, D)
    ncl = np.ascontiguousarray(notes[:R]).astype(BF16)     # (R, D)
    wq = w.astype(BF16)                                    # (D, H)
    nbT = np.ascontiguousarray(nb.T)                       # (D, C)

    # NB natural blocks: nb_sb[p, ct*D + d] = NB[ct*128 + p, d]
    nbp = np.ascontiguousarray(
        nb.reshape(NCT, 128, D).transpose(1, 0, 2).reshape(128, NCT * D))
    wt = np.ascontiguousarray(
        wq.reshape(NDT, 128, H).transpose(1, 0, 2).reshape(128, NDT * H))
    b2 = np.ascontiguousarray(b.reshape(NHT, 128).T)       # (128, NHT) f32

    in_maps = []
    for k in range(NCORES):
        # own body slice (transposed): nbo[p, dt*MSHARD + c] =
        #   NB[k*MSHARD + c, dt*128 + p]
        nbo = np.ascontiguousarray(
            nbT[:, k * MSHARD:(k + 1) * MSHARD]
            .reshape(NDT, 128, MSHARD).transpose(1, 0, 2)
            .reshape(128, NDT * MSHARD))

        nck = ncl[k * MSHARD:(k + 1) * MSHARD]              # (MSHARD, D)
        nct = np.ascontiguousarray(
            nck.T.reshape(NDT, 128, MSHARD).transpose(1, 0, 2)
            .reshape(128, NDT * MSHARD))

        wk = weight[k * MSHARD:(k + 1) * MSHARD].astype(BF16)   # (MSHARD, C)
        # [mc*8+cbp, p(c_local), j*512+n] = wk[mc*512+n, (cbp*8+j)*128+p]
        wpe = np.ascontiguousarray(
            wk.reshape(2, 512, NSLAB, 8, 128).transpose(0, 2, 4, 3, 1)
            .reshape(2 * NSLAB, 128, 8 * 512))

        in_maps.append({
            "nb": nbp, "nbo": nbo, "nct": nct, "wt": wt, "b2": b2,
            "wpe": wpe,
        })
    return in_maps


def kernel(notes, weight, w, b):
    from concourse.bass_utils import run_bass_kernel_spmd

    notes = np.asarray(notes, dtype=np.float32)
    weight = np.asarray(weight, dtype=np.float32)
    w = np.asarray(w, dtype=np.float32)
    b = np.asarray(b, dtype=np.float32)

    nc = _get_nc()
    in_maps = _pack_inputs(notes, weight, w, b)
    res = run_bass_kernel_spmd(nc, in_maps, core_ids=list(range(NCORES)),
                               trace=False)

    out = np.empty((R + 2 * C, H), dtype=np.float32)
    for k in range(NCORES):
        r = res.results[k]
        out[k * MSHARD:(k + 1) * MSHARD] = \
            r["topt_out"].reshape(H, MSHARD).T
        # x0b_out[p, ht*MSHARD + c] = X0own[c, ht*128 + p]
        out[R + C + k * MSHARD:R + C + (k + 1) * MSHARD] = \
            r["x0b_out"].reshape(128, NHT, MSHARD).transpose(2, 1, 0) \
            .reshape(MSHARD, H).astype(np.float32)
    out[R:R + C] = np.maximum(b, 0.0)[None, :]
    return out
